# revision 35
# baseline (speedup 1.0000x reference)
"""BiLSTM layer (B=32, T=512, D=512, H=512) as a Bass/Trainium2 kernel on 8 NeuronCores.

Sharding: 8 cores = 2 directions x 4 batch-quarters. Each core runs a full
forward LSTM scan over T=512 steps for 8 examples of one direction (the
backward direction is realized as a forward scan over time-reversed inputs,
prepared on the host). Weights for that direction are replicated per core.

Mask handling ("ragged" lengths): instead of per-step h/c blending, the mask
is folded into the gate pre-activations (computed in the input-projection
pre-pass): for padded steps t >= len_b the f-gate pre-activation gets +BIG
(sigmoid -> 1) and the i-gate pre-activation gets -BIG (sigmoid -> 0), so c
freezes exactly. h at padded steps is garbage for the forward direction but
is never consumed (all later steps are also padded); the host replaces the
padded forward tail with h[len-1] via a gather. For the backward direction,
c stays 0 through the (reversed) padded prefix, so h = o*tanh(0) = 0 exactly
as the reference requires; no fixup is needed.

The tanh for the g-gate is computed as 2*sigmoid(2z)-1 (the 2z folding is
pre-baked into the g rows of W_ih/W_hh and the g bias), so all four gate
nonlinearities run as a single sigmoid activation per step.
"""

import os
import sys

import numpy as np

sys.path.insert(0, "/opt/trn_rl_repo")

import concourse.bass as bass  # noqa: E402
import concourse.bacc as bacc  # noqa: E402
import concourse.tile as tile  # noqa: E402
from concourse import mybir  # noqa: E402

import ml_dtypes  # noqa: E402

F32 = mybir.dt.float32
F16 = mybir.dt.float16
F8 = mybir.dt.float8e4  # e4m3 — used for W_hh only (4x fast-weight-load)
F8_NP = mybir.dt.np(F8)
AF = mybir.ActivationFunctionType
ALU = mybir.AluOpType

B, D, H = 32, 512, 512
G = 4 * H  # 2048 gate channels
NCORES = 8
BC = 8  # batch per core
KT = D // 128  # 4 k-tiles
MT = G // 128  # 16 m-tiles
WIN = 64  # steps per prepass window
BIG = 60.0

_T_DEFAULT = 512


def _build_nc(T: int, variant: str = "split"):
    """Build the SPMD single-core program (identical on all 8 cores).

    variant: "base"  — single full-width add+sigmoid after all gate matmuls
             "split" — i/f/g gates in one PSUM tile, o in another; the c-path
                       starts while the o-gate matmuls+sigmoid still run
             "nocpath"/"noadd" — diagnostic variants for cost attribution
    """
    nwin = T // WIN
    nc = bacc.Bacc("TRN2", target_bir_lowering=False, debug=False, num_devices=NCORES)

    xT_d = nc.dram_tensor("xT", [D, T * BC], F16, kind="ExternalInput")
    wih_d = nc.dram_tensor("wih", [D, G], F16, kind="ExternalInput")
    whh_d = nc.dram_tensor("whh", [H, G], F8, kind="ExternalInput")
    bias_d = nc.dram_tensor("bias", [128, MT], F32, kind="ExternalInput")
    mb_d = nc.dram_tensor("mb", [128, T * BC], F32, kind="ExternalInput")
    hout_d = nc.dram_tensor("hout", [T, 128, KT * BC], F16, kind="ExternalOutput")

    with tile.TileContext(nc) as tc:
        with (
            tc.tile_pool(name="const", bufs=1) as constp,
            tc.tile_pool(name="xg", bufs=2) as xgp,
            tc.tile_pool(name="xc", bufs=2) as xcp,
            tc.tile_pool(name="mbc", bufs=2) as mbp,
            tc.tile_pool(name="state_h", bufs=4) as hp,
            tc.tile_pool(name="state_c", bufs=2) as cp,
            tc.tile_pool(name="ew", bufs=3) as ewp,
            tc.tile_pool(name="gpsum", bufs=2, space="PSUM") as gpsp,
            tc.tile_pool(name="gopsum", bufs=2, space="PSUM") as gosp,
            tc.tile_pool(name="ppsum", bufs=2, space="PSUM") as ppsp,
        ):
            # ---- persistent weights/bias in SBUF ----
            wih_sb = constp.tile([128, KT, G], F16, tag="wih")
            whh_sb = constp.tile([128, KT, G], F8, tag="whh")
            bias_sb = constp.tile([128, MT], F32, tag="bias")
            for k in range(KT):
                nc.sync.dma_start(wih_sb[:, k, :], wih_d[k * 128:(k + 1) * 128, :])
                nc.sync.dma_start(whh_sb[:, k, :], whh_d[k * 128:(k + 1) * 128, :])
            nc.sync.dma_start(bias_sb[:], bias_d[:])

            # ---- initial state ----
            h_prev = hp.tile([128, KT * BC], F16, tag="h")
            c_prev = cp.tile([128, KT * BC], F32, tag="c")
            nc.vector.memset(h_prev[:], 0.0)
            nc.vector.memset(c_prev[:], 0.0)

            xg_tiles = {}

            # ---- prepass: input projections + bias + mask-bias for one window ----
            def make_prepass(w):
                xg_w = xgp.tile([128, WIN, 128], F32, tag="xg")
                xc = xcp.tile([128, KT, WIN * BC], F16, tag="xc")
                mbc = mbp.tile([128, WIN, BC], F32, tag="mbc")
                xg_tiles[w] = xg_w
                c0 = w * WIN * BC
                chunks = []

                def dma_in():
                    for k in range(KT):
                        nc.sync.dma_start(
                            xc[:, k, :], xT_d[k * 128:(k + 1) * 128, c0:c0 + WIN * BC]
                        )
                    nc.sync.dma_start(
                        mbc[:],
                        mb_d[:, c0:c0 + WIN * BC].rearrange("p (s b) -> p s b", b=BC),
                    )

                chunks.append(dma_in)

                def mtile(m):
                    pre = ppsp.tile([128, WIN * BC], F32, tag="pre")
                    for k in range(KT):
                        nc.tensor.matmul(
                            pre[:],
                            wih_sb[:, k, m * 128:(m + 1) * 128],
                            xc[:, k, :],
                            start=(k == 0),
                            stop=(k == KT - 1),
                        )
                    nc.scalar.activation(
                        xg_w[:, :, m * BC:(m + 1) * BC],
                        pre[:].rearrange("p (s b) -> p s b", b=BC),
                        AF.Identity,
                        bias=bias_sb[:, m:m + 1],
                    )
                    if m < KT:
                        nc.vector.tensor_tensor(
                            xg_w[:, :, m * BC:(m + 1) * BC],
                            xg_w[:, :, m * BC:(m + 1) * BC],
                            mbc[:],
                            ALU.subtract,
                        )
                    elif m < 2 * KT:
                        nc.vector.tensor_tensor(
                            xg_w[:, :, m * BC:(m + 1) * BC],
                            xg_w[:, :, m * BC:(m + 1) * BC],
                            mbc[:],
                            ALU.add,
                        )

                for m in range(MT):
                    chunks.append(lambda m=m: mtile(m))
                return chunks

            # prologue: window 0 fully
            for fn in make_prepass(0):
                fn()

            pending = []  # chunk closures of the next window, drip-fed
            n_issued = 0
            for t in range(T):
                w, s = divmod(t, WIN)
                if s == 0:
                    assert not pending, f"window {w}: {len(pending)} chunks undrained"
                    if w + 1 < nwin:
                        pending = make_prepass(w + 1)
                        n_issued = 0
                xg_w = xg_tiles[w]

                ncol = KT * BC  # 32
                drip = []
                if pending:
                    want = (s + 1) * (1 + MT) // WIN
                    while n_issued < want and pending:
                        drip.append(pending.pop(0))
                        n_issued += 1

                if variant == "split":
                    # i/f/g gate matmuls into one PSUM tile, o into another
                    # (separate banks -> the ifg add/sigmoid and the c-path
                    # overlap the o-tile matmuls instead of serializing on
                    # the PSUM bank).
                    g_ifg = gpsp.tile([128, 12 * BC], F32, tag="gates")
                    g_o = gosp.tile([128, 4 * BC], F32, tag="gates_o")
                    for m in range(12):
                        for k in range(KT):
                            nc.tensor.matmul(
                                g_ifg[:, m * BC:(m + 1) * BC],
                                whh_sb[:, k, m * 128:(m + 1) * 128],
                                h_prev[:, k * BC:(k + 1) * BC],
                                start=(k == 0),
                                stop=(k == KT - 1),
                            )
                    pre_ifg = ewp.tile([128, 12 * BC], F32, tag="gpre")
                    nc.vector.tensor_add(pre_ifg[:], g_ifg[:], xg_w[:, s, 0:96])
                    sig_ifg = ewp.tile([128, 12 * BC], F32, tag="gsig")
                    nc.scalar.activation(sig_ifg[:], pre_ifg[:], AF.Sigmoid)
                    for m in range(12, MT):
                        for k in range(KT):
                            nc.tensor.matmul(
                                g_o[:, (m - 12) * BC:(m - 11) * BC],
                                whh_sb[:, k, m * 128:(m + 1) * 128],
                                h_prev[:, k * BC:(k + 1) * BC],
                                start=(k == 0),
                                stop=(k == KT - 1),
                            )
                    for fn in drip:
                        fn()
                    pre_o = ewp.tile([128, 4 * BC], F32, tag="gpre_o")
                    nc.vector.tensor_add(pre_o[:], g_o[:], xg_w[:, s, 96:128])
                    sig_o = ewp.tile([128, 4 * BC], F32, tag="gsig_o")
                    nc.scalar.activation(sig_o[:], pre_o[:], AF.Sigmoid)
                    i_s = sig_ifg[:, 0 * ncol:1 * ncol]
                    f_s = sig_ifg[:, 1 * ncol:2 * ncol]
                    g_s = sig_ifg[:, 2 * ncol:3 * ncol]
                    o_s = sig_o[:]
                else:
                    gates = gpsp.tile([128, MT * BC], F32, tag="gates")
                    for m in range(MT):
                        for k in range(KT):
                            nc.tensor.matmul(
                                gates[:, m * BC:(m + 1) * BC],
                                whh_sb[:, k, m * 128:(m + 1) * 128],
                                h_prev[:, k * BC:(k + 1) * BC],
                                start=(k == 0),
                                stop=(k == KT - 1),
                            )
                    for fn in drip:
                        fn()
                    gsig = ewp.tile([128, MT * BC], F32, tag="gsig")
                    if variant == "noadd":
                        nc.scalar.activation(gsig[:], gates[:], AF.Sigmoid)
                    else:
                        gpre = ewp.tile([128, MT * BC], F32, tag="gpre")
                        nc.vector.tensor_add(gpre[:], gates[:], xg_w[:, s, :])
                        nc.scalar.activation(gsig[:], gpre[:], AF.Sigmoid)
                    i_s = gsig[:, 0 * ncol:1 * ncol]
                    f_s = gsig[:, 1 * ncol:2 * ncol]
                    g_s = gsig[:, 2 * ncol:3 * ncol]
                    o_s = gsig[:, 3 * ncol:4 * ncol]

                if variant == "nocpath":
                    h_new = hp.tile([128, ncol], F16, tag="h")
                    nc.vector.tensor_mul(h_new[:], o_s, i_s)
                    c_new = c_prev
                else:
                    # i*tanh(z_g) = 2*((sigmoid(2 z_g) - 0.5) * i): two fused
                    # scalar_tensor_tensor ops replace gfix -> ig -> csum.
                    fc = ewp.tile([128, ncol], F32, tag="fc")
                    nc.vector.tensor_mul(fc[:], f_s, c_prev[:])
                    t1 = ewp.tile([128, ncol], F32, tag="ig")
                    nc.vector.scalar_tensor_tensor(
                        t1[:], g_s, 0.5, i_s, ALU.subtract, ALU.mult
                    )
                    c_new = cp.tile([128, ncol], F32, tag="c")
                    nc.vector.scalar_tensor_tensor(
                        c_new[:], t1[:], 2.0, fc[:], ALU.mult, ALU.add
                    )
                    tc_t = ewp.tile([128, ncol], F32, tag="tanh_c")
                    nc.scalar.activation(tc_t[:], c_new[:], AF.Tanh)
                    h_new = hp.tile([128, ncol], F16, tag="h")
                    nc.vector.tensor_mul(h_new[:], o_s, tc_t[:])

                nc.sync.dma_start(hout_d[t], h_new[:])

                h_prev, c_prev = h_new, c_new

    nc.compile()
    return nc


_NC_CACHE = {}


def _get_nc(T, variant=None):
    variant = variant or os.environ.get("BASS_LSTM_VARIANT", "split")
    key = (T, variant)
    if key not in _NC_CACHE:
        _NC_CACHE[key] = _build_nc(T, variant)
    return _NC_CACHE[key]


_RUNNER_CACHE = {}


def _get_runner(nc):
    """Compile the SPMD executable once per program; reuse across calls.

    Forked from concourse.bass2jax.run_bass_via_pjrt (the @via_axon
    redirect target), minus the NTFF-trace path (unavailable here) and
    with the jitted callable cached so repeat kernel() calls skip the
    multi-minute walrus compile.
    """
    if id(nc) in _RUNNER_CACHE:
        return _RUNNER_CACHE[id(nc)]
    import jax
    from jax.sharding import Mesh, PartitionSpec
    from jax.experimental.shard_map import shard_map
    from concourse import bass2jax

    bass2jax.install_neuronx_cc_hook()

    partition_name = (
        nc.partition_id_tensor.name if nc.partition_id_tensor is not None else None
    )
    in_names, out_names, out_avals, zero_shapes = [], [], [], []
    for alloc in nc.m.functions[0].allocations:
        if not isinstance(alloc, mybir.MemoryLocationSet):
            continue
        name = alloc.memorylocations[0].name
        if alloc.kind == "ExternalInput":
            if name != partition_name:
                in_names.append(name)
        elif alloc.kind == "ExternalOutput":
            shape = tuple(alloc.tensor_shape)
            dtype = mybir.dt.np(alloc.dtype)
            out_names.append(name)
            out_avals.append(jax.core.ShapedArray(shape, dtype))
            zero_shapes.append((shape, dtype))
    n_params = len(in_names)
    all_in_names = in_names + out_names
    if partition_name is not None:
        all_in_names = all_in_names + [partition_name]

    def _body(*args):
        operands = list(args)
        if partition_name is not None:
            operands.append(bass2jax.partition_id_tensor())
        outs = bass2jax._bass_exec_p.bind(
            *operands,
            out_avals=tuple(out_avals),
            in_names=tuple(all_in_names),
            out_names=tuple(out_names),
            lowering_input_output_aliases=(),
            sim_require_finite=True,
            sim_require_nnan=True,
            nc=nc,
        )
        return tuple(outs)

    devices = jax.devices()[:NCORES]
    mesh = Mesh(np.asarray(devices), ("core",))
    nspecs = n_params + len(out_names)
    sharded = jax.jit(
        shard_map(
            _body,
            mesh=mesh,
            in_specs=(PartitionSpec("core"),) * nspecs,
            out_specs=(PartitionSpec("core"),) * len(out_names),
            check_rep=False,
        ),
        donate_argnums=tuple(range(n_params, nspecs)),
        keep_unused=True,
    )
    runner = (sharded, in_names, out_names, out_avals, zero_shapes)
    _RUNNER_CACHE[id(nc)] = runner
    return runner


def _run_spmd(nc, in_maps):
    sharded, in_names, out_names, out_avals, zero_shapes = _get_runner(nc)
    concat_in = [
        np.concatenate([np.asarray(in_maps[c][name]) for c in range(NCORES)], axis=0)
        for name in in_names
    ]
    concat_zeros = [
        np.zeros((NCORES * s[0], *s[1:]), dt) for (s, dt) in zero_shapes
    ]
    import time as _time

    t0 = _time.perf_counter()
    out_arrs = sharded(*concat_in, *concat_zeros)
    out_arrs = [np.asarray(a) for a in out_arrs]
    _run_spmd.last_wall_s = _time.perf_counter() - t0
    return [
        {
            name: out_arrs[i].reshape(NCORES, *out_avals[i].shape)[c]
            for i, name in enumerate(out_names)
        }
        for c in range(NCORES)
    ]


_run_spmd.last_wall_s = None


def _prep_core_inputs(x, lengths, wih, whh, bsum, q, reverse, T):
    """Host-side input prep for one core (batch quarter q, one direction)."""
    xs = x[q * BC:(q + 1) * BC, :, :]  # [BC, T, D]
    ls = lengths[q * BC:(q + 1) * BC]  # [BC]
    if reverse:
        xs = xs[:, ::-1, :]
    xT = np.ascontiguousarray(xs.transpose(2, 1, 0).reshape(D, T * BC))
    mask = (ls[None, :] > np.arange(T)[:, None]).astype(np.float32)  # [T, BC]
    if reverse:
        mask = mask[::-1]
    mb = BIG * (1.0 - mask)  # [T, BC]
    mb128 = np.ascontiguousarray(
        np.broadcast_to(mb.reshape(1, T * BC), (128, T * BC))
    ).astype(np.float32)
    return {
        "xT": xT.astype(np.float16),
        "wih": wih,
        "whh": whh,
        "bias": bsum,
        "mb": mb128,
    }


def _prep_direction_weights(W_ih, W_hh, b_ih, b_hh):
    wih = np.ascontiguousarray(W_ih.T).astype(np.float32).copy()  # [D, G]
    whh = np.ascontiguousarray(W_hh.T).astype(np.float32).copy()  # [H, G]
    bsum = (b_ih + b_hh).astype(np.float32).copy()  # [G]
    # fold the tanh-via-sigmoid 2x into the g-gate block (cols 2H:3H)
    wih[:, 2 * H:3 * H] *= 2.0
    whh[:, 2 * H:3 * H] *= 2.0
    bsum[2 * H:3 * H] *= 2.0
    bias = np.ascontiguousarray(bsum.reshape(MT, 128).T).astype(np.float32)  # [128, MT]
    return (
        wih.astype(np.float16),
        whh.astype(F8_NP),
        bias,
    )


def _assemble_direction(houts, lengths, T, reverse):
    """houts: list of 4 per-quarter [T, 128, KT*BC] arrays -> [B, T, H] f32."""
    hs = []
    for q in range(4):
        h = np.asarray(houts[q]).astype(np.float32)  # [T, 128, 32]
        h = h.reshape(T, 128, KT, BC).transpose(3, 0, 2, 1).reshape(BC, T, H)
        hs.append(h)
    h = np.concatenate(hs, axis=0)  # [B, T, H]
    if reverse:
        h = h[:, ::-1, :]
    else:
        # replace padded tail with h[len-1]
        idx = np.minimum(np.arange(T)[None, :], (lengths - 1)[:, None])  # [B, T]
        h = h[np.arange(B)[:, None], idx]
    return h


def kernel(x, lengths, W_ih_f, W_hh_f, b_ih_f, b_hh_f, W_ih_b, W_hh_b, b_ih_b, b_hh_b):
    T = x.shape[1]
    x = np.asarray(x, dtype=np.float32)
    lengths = np.asarray(lengths).astype(np.int64)

    wih_f, whh_f, bias_f = _prep_direction_weights(W_ih_f, W_hh_f, b_ih_f, b_hh_f)
    wih_b, whh_b, bias_b = _prep_direction_weights(W_ih_b, W_hh_b, b_ih_b, b_hh_b)

    in_maps = []
    for r in range(NCORES):
        reverse = r >= 4
        q = r % 4
        if reverse:
            m = _prep_core_inputs(x, lengths, wih_b, whh_b, bias_b, q, True, T)
        else:
            m = _prep_core_inputs(x, lengths, wih_f, whh_f, bias_f, q, False, T)
        in_maps.append(m)

    nc = _get_nc(T)
    results = _run_spmd(nc, in_maps)
    kernel.last_wall_s = _run_spmd.last_wall_s

    h_f = _assemble_direction(
        [results[r]["hout"] for r in range(4)], lengths, T, reverse=False
    )
    h_b = _assemble_direction(
        [results[r]["hout"] for r in range(4, 8)], lengths, T, reverse=True
    )
    return np.concatenate([h_f, h_b], axis=-1).astype(np.float32)


kernel.last_exec_time_ns = None
kernel.last_wall_s = None
